# revision 36
# baseline (speedup 1.0000x reference)
"""Trainium2 Bass kernel for nn_ExactSpectralHead (sparse resonance attention).

Reference computation (per batch element b):
    q = x @ Wq.T; k = x @ Wk.T; v = x @ Wv.T          # [T, H]
    s = (q @ k.T) * C**-0.5 + resonance_bias          # [T, T]
    s = where(allowed, s, -inf); p = softmax(s, -1)
    out = p @ v                                        # [T, H]

Strategy (8 NeuronCores, data-parallel over batch B=8, one b per core):
  - Host folds bias+mask into EB = exp(bias) * allowed (exact: exp(log1p(r)) = 1+r),
    so p_raw = exp(s_qk * scale) * EB with no -inf handling and exact zeros.
    Scores are bounded (|s|<~5), so no max-subtraction is needed; normalization
    (division by the row sum) is done on the HOST from the unnormalized PV
    output plus a row-sum computed on-device via a ones-matmul.
  - Everything is computed in a transposed layout so that every matmul contracts
    over the partition dim with zero on-device transposes:
      xT [C, T] (host-transposed), QT/KT = W.T^T @ xT -> [H, T],
      ST[tk, tq] = KT.T @ QT, PT = exp(ST*scale) * EBT,
      OT[h, tq] += V[tk,:].T @ PT[tk, tq]   (V in natural [T, H] layout),
      rowsum[tq] = ones.T @ sum_i PT_i, out = (OT / rowsum).T (host).
  - Q/K projections use fp8e4 inputs with DoubleRow matmuls (two 128-deep
    contraction chunks per pass -> 2x PE throughput). The score noise this
    introduces is ~0.3% absolute on s (scores are tiny vs the bias), well
    inside the 2e-2 tolerance. V stays bf16 (fp8 V noise would land ~1:1 on
    the output).
  - bf16 matmul inputs elsewhere (1 col/cycle on the PE), fp32 PSUM accum.
  - Causal block skipping: tiles with tk_chunk > tq_block are never touched.
  - The PE p-state ramps to 2.4GHz only after ~3us of continuous work, so a
    burst of dummy warmup matmuls runs during the initial DMA wait.
"""

import sys

sys.path.insert(0, "/opt/trn_rl_repo")

import numpy as np
import ml_dtypes

import concourse.bass as bass
import concourse.tile as tile
import concourse.mybir as mybir

# ----------------------------------------------------------------------------
# Workaround for walrus codegen "Too many sync wait commands" on the
# TileContext tail Drain: split the global-clock sem waits across multiple SP
# NOP instructions instead of attaching them all to the single Drain.
from concourse.vector_clock import ScopedClock, VectorClock


def _split_drain_and_barrier(self, tick_clock, wait_clock):
    """Cheap kernel tail: per-proc sem waits split across SP NOPs (walrus
    one-wait-per-instruction limit), then a regular-semaphore all-engine
    completion barrier (the stock EVSEM butterfly costs ~1.5-4us per hop),
    then GpSimd clears the tile semaphores. The next NEFF execution cannot
    start until every engine stream (including the clear) retires, so no
    trailing barrier is needed."""
    import concourse.mybir as _mybir

    nc = self.nc
    gc = tick_clock.global_clock
    n = len(gc)
    for p in range(n):
        t = gc[p]
        if t > 0:
            nop = nc.sync.nop(nofuse=True, hint=f"drain_wait_{p}")
            vc = VectorClock([t if i == p else 0 for i in range(n)])
            wait_clock.add_sem_waits(nop.ins, ScopedClock({None: vc}))

    tail_sem = nc.alloc_semaphore("tile_tail_sem")
    n_signals = 0
    for etype, eng in nc.engines.items():
        if etype == _mybir.EngineType.Pool:
            continue
        eng.drain(fusable=False)
        eng.sem_inc(tail_sem, 1)
        n_signals += 1
    nc.gpsimd.wait_ge(tail_sem, n_signals)
    assert self.sems is not None
    popped = nc._tile_sem_poison_stack.pop()
    assert popped is self._sem_poison
    nc.clear_and_free_semaphores(list(self.sems.allocated().values()))
    nc.gpsimd.sem_clear(range(tail_sem.num, tail_sem.num + 1))


tile.TileContext._drain_and_barrier = _split_drain_and_barrier
# ----------------------------------------------------------------------------

def _split_excess_waits(nc, max_waits=1):
    """Walrus codegen in this toolchain supports only one sem-wait per
    instruction; hoist excess waits onto preceding same-engine NOPs."""
    for f in nc.m.functions:
        for bb in f.blocks:
            new = []
            changed = False
            for inst in bb.instructions:
                if isinstance(inst, mybir.InstEventSemaphore):
                    # EventSemaphore ops measure ~3-5us on HW; their barrier
                    # semantics live entirely in sync_info (regular sems), so
                    # NoOps with the same sync_info are equivalent and fast.
                    # Waits and updates go on separate NoOps (wait first) to
                    # satisfy the no_semaphore_value_conflict ISA check.
                    si = inst.sync_info
                    changed = True
                    w = list(si.on_wait) if si else []
                    u = list(si.on_update) if si else []
                    if w:
                        new.append(
                            mybir.InstNoOp(
                                name=f"{inst.name}-wait",
                                engine=inst.engine,
                                bass_nofuse=True,
                                sync_info=mybir.SyncInfo(on_wait=w, on_update=[]),
                            )
                        )
                    new.append(
                        mybir.InstNoOp(
                            name=inst.name,
                            engine=inst.engine,
                            bass_nofuse=True,
                            sync_info=mybir.SyncInfo(on_wait=[], on_update=u),
                        )
                    )
                    continue
                si = inst.sync_info
                waits = list(si.on_wait) if si is not None else []
                if len(waits) > max_waits:
                    changed = True
                    excess, keep = waits[:-max_waits], waits[-max_waits:]
                    for k, w in enumerate(excess):
                        new.append(
                            mybir.InstNoOp(
                                name=f"{inst.name}-w{k}",
                                engine=inst.engine,
                                bass_nofuse=True,
                                sync_info=mybir.SyncInfo(on_wait=[w], on_update=[]),
                            )
                        )
                    inst.sync_info = mybir.SyncInfo(
                        on_wait=keep, on_update=list(si.on_update)
                    )
                new.append(inst)
            if changed:
                bb.instructions = new


B, T, C, H = 8, 2048, 1024, 128
NCORES = 8
QS = 16.0                # host-side Wq/Wk scale (lifts fp8 weights out of subnormals)
WS = 64.0                # host-side Wv scale (ditto; host divides the output by it)
SCALE = float(C) ** -0.5 / (QS * QS)   # exp() scale absorbs the q/k scaling
P = 128
TQ = 512                 # tq block width (matmul moving dim)
NJ = T // TQ             # 4 tq blocks
NC_CHUNK = C // P        # 8 contraction chunks over channels
NCP = NC_CHUNK // 2      # 4 DoubleRow c-chunk pairs
NK = T // P              # 16 tk chunks
BF16 = mybir.dt.bfloat16
FP8 = mybir.dt.float8e4
F32 = mybir.dt.float32
DR = mybir.MatmulPerfMode.DoubleRow

_nc_cache = None


def _build_nc():
    nc = bass.Bass()
    # fp8 DoubleRow pack of xT: x8[j, p, cp, i, q] = xT[(2cp+i)*128+p, j*TQ+q]
    # (partition-major so each DMA moves 4KB contiguous per partition)
    x8t = nc.declare_dram_parameter("x8t", [NJ, P, NCP, 2, TQ], FP8, isOutput=False)
    # fp8 residual pack: xr8 = fp8(xT - fp8(xT)); x8+xr8 represents x to ~0.13%
    xr8t = nc.declare_dram_parameter("xr8t", [NJ, P, NCP, 2, TQ], FP8, isOutput=False)
    # fp8 DoubleRow packs of all weights (one DMA), partition-major:
    # w8all[p, s, cp, i, h] = W_s[h, (2cp+i)*128+p], s = (Wq, Wk, Wv, Wv_resid)
    w8all = nc.declare_dram_parameter("w8all", [P, 4, NCP, 2, H], FP8, isOutput=False)
    # ebT packed partition-major, j-grouped quads (quad (j,q4) at slot off_j+q4):
    # ebp[p, off_j + q4, k, q] = EB.T[128*(4*q4+k)+p, j*TQ+q]; one DMA per j.
    # bf16: a fp8 operand halves the DVE mul rate, so spend the DMA bytes
    ebp = nc.declare_dram_parameter("ebp", [P, 10, 4, TQ], BF16, isOutput=False)
    # unnormalized PV output, bf16: [j, H, TQ] (host divides by rowsum + reassembles)
    outt = nc.declare_dram_parameter("outt", [NJ, H, TQ], BF16, isOutput=True)
    # rowsums, fp32: [j, 1, TQ]
    rsum = nc.declare_dram_parameter("rsum", [NJ, 1, TQ], F32, isOutput=True)

    with tile.TileContext(nc) as tc:
        with (
            tc.tile_pool(name="const", bufs=1) as const,
            tc.tile_pool(name="qkv_psum", bufs=2, space="PSUM") as qkv_psum,
            tc.tile_pool(name="st_psum", bufs=2, space="PSUM") as st_psum_pool,
            tc.tile_pool(name="ot_psum", bufs=1, space="PSUM") as ot_psum_pool,
            tc.tile_pool(name="rs_psum", bufs=1, space="PSUM") as rs_psum_pool,
            tc.tile_pool(name="pt", bufs=NK // 2 + 4) as pt_pool,
            tc.tile_pool(name="eb", bufs=1) as eb_pool,
            tc.tile_pool(name="outs", bufs=4) as out_pool,
        ):
            # ---------- persistent SBUF tensors ----------
            # warmup garbage tile: ramp the PE p-state while DMAs land.
            warm = const.tile([P, P], BF16, tag="warm", name="warm_sb")
            warm_ps = qkv_psum.tile([P, TQ], F32, tag="qkvps", name="qkvps")
            nc.vector.memset(warm[:], 0.0)
            for i in range(96):
                nc.tensor.matmul(
                    warm_ps[:, :P], lhsT=warm[:], rhs=warm[:],
                    start=True, stop=True, skip_group_check=True,
                )

            w8all_sb = const.tile([P, 4, NCP, 2, H], FP8, tag="w8all", name="w8all_sb")
            x8_sb = const.tile([P, NJ, NCP, 2, TQ], FP8, tag="x8", name="x8_sb")
            xr8_sb = const.tile([P, NJ, NCP, 2, TQ], FP8, tag="xr8", name="xr8_sb")
            # per-j eb tiles (all quads of a block in one tile, one DMA)
            ebjs = [
                eb_pool.tile([P, (j + 1) * 4, TQ], BF16, tag=f"eb{j}", name=f"eb{j}")
                for j in range(NJ)
            ]
            EBOFF = [0, 1, 3, 6]

            # ---------- all DMAs issued up-front, in consumption order.   ----------
            # Few, large transfers: the DMA queues have a multi-us fixed cost
            # per transfer. Per-queue lists are in global consumption order;
            # start-critical bulk rides the fast sync/gpsimd queues, the slow
            # scalar queue gets only mid/late items.
            def load_x8(j, eng):
                eng.dma_start(x8_sb[:, j], x8t[j])

            def load_xr8(j, eng):
                eng.dma_start(xr8_sb[:, j], xr8t[j])

            def load_eb(j, eng):
                eng.dma_start(ebjs[j][:], ebp[:, EBOFF[j]:EBOFF[j] + j + 1])

            # need-ordered per queue; eb2 is issued later from the scalar
            # stream (inside the j=0 attention loop) so its bulk does not
            # steal early bandwidth. scalar also carries the outputs.
            # The start-critical path (w8q + x8j0 for the first QT matmuls)
            # is split into small leading transfers.
            nc.sync.dma_start(w8all_sb[:, 0:1], w8all[:, 0:1])   # w8q, 64KB
            nc.sync.dma_start(x8_sb[:, 0, 0:2], x8t[0, :, 0:2])  # 256KB
            nc.sync.dma_start(x8_sb[:, 0, 2:4], x8t[0, :, 2:4])  # 256KB
            nc.gpsimd.dma_start(w8all_sb[:, 1:4], w8all[:, 1:4]) # wk/wv/wvr
            load_xr8(0, nc.gpsimd)                      # 512KB
            load_eb(0, nc.sync)                         # 512KB
            load_x8(1, nc.gpsimd)                       # 512KB
            load_xr8(1, nc.sync)                        # 512KB
            load_eb(1, nc.gpsimd)                       # 1MB
            load_x8(2, nc.sync)                         # 512KB
            load_xr8(2, nc.gpsimd)                      # 512KB
            load_x8(3, nc.gpsimd)                       # 512KB
            load_xr8(3, nc.sync)                        # 512KB
            load_eb(3, nc.gpsimd)                       # 2MB

            QT_sb = const.tile([P, T], BF16, tag="QT", name="QT_sb")
            KT_sb = const.tile([P, T], BF16, tag="KT", name="KT_sb")
            v_sb = const.tile([P, NK, H], BF16, tag="V", name="v_sb")
            ones_sb = const.tile([P, P], BF16, tag="ones", name="ones_sb")
            nc.vector.memset(ones_sb[:], 1.0)
            ones8_sb = const.tile([P, 2, P], FP8, tag="ones8", name="ones8_sb")
            nc.vector.memset(ones8_sb[:], 1.0)

            # ---------- interleaved per tq-block: V_(4j..4j+3), ATT_j; ----------
            # ---------- QT/KT for block j+1 are emitted inside block j's ----------
            # ---------- trailing section to hide the exp/mul chase there ----------
            def emit_qkt(j):
                for s, dst in ((0, QT_sb), (1, KT_sb)):
                    ps = qkv_psum.tile([P, TQ], F32, tag="qkvps", name="qkvps")
                    for cp in range(NCP):
                        nc.tensor.matmul(
                            ps[:],
                            lhsT=w8all_sb[:, s, cp],
                            rhs=x8_sb[:, j, cp],
                            start=(cp == 0),
                            stop=(cp == NCP - 1),
                            perf_mode=DR,
                        )
                    nc.vector.tensor_copy(dst[:, j * TQ:(j + 1) * TQ], ps[:])

            def emit_v(j):
                # V chunks 4j .. 4j+3: compensated fp8 DoubleRow,
                # v = (x8 + xr8) @ w8v + x8 @ w8vr  (~0.15% accurate)
                for mq in range(4):
                    m = 4 * j + mq
                    msl = slice(mq * P, (mq + 1) * P)
                    ps = qkv_psum.tile([P, TQ], F32, tag="qkvps", name="qkvps")
                    first = True
                    for xsb, ws in ((x8_sb, 2), (xr8_sb, 2), (x8_sb, 3)):
                        for cp in range(NCP):
                            nc.tensor.matmul(
                                ps[:, :H],
                                lhsT=xsb[:, j, cp, :, msl],
                                rhs=w8all_sb[:, ws, cp],
                                start=first,
                                stop=(ws == 3 and cp == NCP - 1),
                                perf_mode=DR,
                            )
                            first = False
                    nc.vector.tensor_copy(v_sb[:, m, :], ps[:, :H])

            emit_qkt(0)
            for j in range(NJ):
                emit_v(j)

                # attention for tq block j (causal: tk chunks 0 .. 4j+3).
                # st pairs are emitted ahead; ot/rs matmuls for pair p are
                # emitted after st pair p+2, so the PE never waits on the
                # ACT(exp) -> DVE(mul) chase.
                n_i = 4 * j + 4
                n2 = n_i // 2
                ot = ot_psum_pool.tile([P, TQ], F32, tag="ot", name="ot")
                rs = rs_psum_pool.tile([P, TQ], F32, tag="rs", name="rs")
                pts = []
                pt8s = []

                def emit_ot(p):
                    for k in range(2):
                        i = 2 * p + k
                        o = coff(i)
                        nc.tensor.matmul(
                            ot[:, o:],
                            lhsT=v_sb[:, i, :],
                            rhs=pts[p][:, k, o:],
                            start=(i == 0),
                            stop=(i == n_i - 1),
                            skip_group_check=True,
                        )

                def emit_rs(p):
                    if p >= n2 - 2:
                        # diagonal quad in bf16: sparse queries (prime-ish
                        # positions) have all their softmax mass here, and a
                        # num/den precision mismatch would not cancel for them
                        for k in range(2):
                            i = 2 * p + k
                            o = coff(i)
                            nc.tensor.matmul(
                                rs[:, o:],
                                lhsT=ones_sb[:],
                                rhs=pts[p][:, k, o:],
                                start=(i == 0),
                                stop=(i == n_i - 1),
                                skip_group_check=True,
                            )
                    else:
                        # off-diagonal pairs: fp8 DoubleRow, one pass per pair;
                        # their noise averages out over the wide support
                        nc.tensor.matmul(
                            rs[:],
                            lhsT=ones8_sb[:],
                            rhs=pt8s[p][:],
                            start=(p == 0),
                            stop=False,
                            perf_mode=DR,
                            skip_group_check=True,
                        )

                def emit_otrs(p):
                    emit_ot(p)
                    emit_rs(p)

                def coff(i):
                    # causal offset: tk chunk i only attends tq >= (i-4j)*128
                    return max(0, (i - 4 * j) * P)

                for p in range(n2):
                    st2 = st_psum_pool.tile([P, 2, TQ], F32, tag="st", name="st2")
                    for k in range(2):
                        i = 2 * p + k
                        o = coff(i)
                        nc.tensor.matmul(
                            st2[:, k, o:],
                            lhsT=KT_sb[:, i * P:(i + 1) * P],
                            rhs=QT_sb[:, j * TQ + o:(j + 1) * TQ],
                            start=True,
                            stop=True,
                        )
                    pt = pt_pool.tile([P, 2, TQ], BF16, tag="pt", name="pt")
                    if j == NJ - 1 or p >= n2 - 2:
                        for k in range(2):
                            nc.scalar.activation(
                                pt[:, k, :], st2[:, k, :],
                                mybir.ActivationFunctionType.Exp, scale=SCALE,
                            )
                            nc.vector.tensor_mul(
                                pt[:, k, :], pt[:, k, :],
                                ebjs[j][:, 2 * p + k, :],
                            )
                    else:
                        nc.scalar.activation(
                            pt[:], st2[:], mybir.ActivationFunctionType.Exp,
                            scale=SCALE,
                        )
                        nc.vector.tensor_mul(
                            pt[:], pt[:], ebjs[j][:, 2 * p:2 * p + 2, :]
                        )
                    pts.append(pt)
                    if p < n2 - 2:
                        pt8 = pt_pool.tile([P, 2, TQ], FP8, tag="pt8", name="pt8")
                        nc.vector.tensor_copy(pt8[:], pt[:])
                        pt8s.append(pt8)
                    else:
                        pt8s.append(None)
                    if j == 0 and p == 1:
                        # artificial WAW dep delays the bulky eb2 transfer so it
                        # cannot steal bandwidth from the start-critical loads
                        nc.vector.tensor_copy(
                            ebjs[2][0:1, 0:1, 0:1], pt[0:1, 0:1, 0:1]
                        )
                        load_eb(2, nc.scalar)           # 1.5MB, deferred issue
                    if p >= 2:
                        emit_otrs(p - 2)
                # next block's projections slot in while the trailing
                # exp/mul chain for the last pairs drains
                if j + 1 < NJ:
                    emit_qkt(j + 1)
                # trailing pairs: rowsum matmuls first so the rs DMA can go
                # out while the PE finishes the ot matmuls
                for p in range(max(0, n2 - 2), n2):
                    emit_rs(p)
                for p in range(max(0, n2 - 2), n2):
                    emit_ot(p)

                # rowsum out (all 128 psum partitions hold the same sums; DMA row 0)
                rsb = out_pool.tile([1, TQ], F32, tag="rsb", name="rsb")
                nc.vector.tensor_copy(rsb[:], rs[0:1, :])
                nc.scalar.dma_start(rsum[j][:], rsb[:])
                # unnormalized OT out, bf16; last block in halves to overlap
                # the copy with the DMA on the tail
                otb = out_pool.tile([P, TQ], BF16, tag="otb", name="otb")
                if j == NJ - 1:
                    HQ = TQ // 2
                    for hh in range(2):
                        sl = slice(hh * HQ, (hh + 1) * HQ)
                        nc.vector.tensor_copy(otb[:, sl], ot[:, sl])
                        nc.scalar.dma_start(outt[j][:, sl], otb[:, sl])
                else:
                    nc.vector.tensor_copy(otb[:], ot[:])
                    nc.scalar.dma_start(outt[j][:], otb[:])

    _split_excess_waits(nc)
    return nc


def _get_nc():
    global _nc_cache
    if _nc_cache is None:
        _nc_cache = _build_nc()
    return _nc_cache


def kernel(x, Wq, Wk, Wv, resonance_bias, allowed):
    x = np.asarray(x, dtype=np.float32)
    Wq = np.asarray(Wq, dtype=np.float32)
    Wk = np.asarray(Wk, dtype=np.float32)
    Wv = np.asarray(Wv, dtype=np.float32)
    resonance_bias = np.asarray(resonance_bias, dtype=np.float32)
    allowed = np.asarray(allowed)

    bf16 = ml_dtypes.bfloat16
    f8 = ml_dtypes.float8_e4m3
    eb = np.exp(resonance_bias) * allowed  # exp(log1p(r))*mask = (1+r)*mask, exact
    ebT = eb.T.astype(bf16)                              # [tk, tq]
    # partition-major, j-grouped quads: ebp[p, EBOFF[j]+q4, k, q]
    #   = EB.T[128*(4*q4+k)+p, j*TQ+q]
    ebq5 = ebT.reshape(NK // 4, 4, P, NJ, TQ)            # [q4, k, p, j, q]
    ebp = np.empty((P, 10, 4, TQ), dtype=bf16)
    EBOFF = [0, 1, 3, 6]
    for j in range(NJ):
        for q4 in range(j + 1):
            ebp[:, EBOFF[j] + q4] = ebq5[q4, :, :, j, :].transpose(1, 0, 2)
    ebp = np.ascontiguousarray(ebp)
    # fp8 DoubleRow weight packs, partition-major, (q, k, v, v_resid) stacked:
    # w8all[p, s, cp, i, h] = W_s[h, (2cp+i)*128+p]. Weights are pre-scaled on
    # the host (QS for q/k, WS for v) so their fp8 encodings avoid the
    # subnormal range; the v residual pack makes v accurate to ~4e-4.
    wv64 = Wv * WS
    wv8 = wv64.astype(f8).astype(np.float32)
    wvr = wv64 - wv8
    w8all = np.ascontiguousarray(
        np.stack(
            [W.T.reshape(NCP, 2, P, H).transpose(2, 0, 1, 3).astype(f8)
             for W in (Wq * QS, Wk * QS, wv8, wvr)],
            axis=1,
        )
    )

    in_maps = []
    for b in range(NCORES):
        xT = x[b].T  # [C, T] fp32
        # x8[j, p, cp, i, q] = xT[(2cp+i)*128+p, j*TQ+q]; xr8 = fp8 residual
        x8 = xT.astype(f8)
        xr8 = (xT - x8.astype(np.float32)).astype(f8)
        def pack(a):
            return np.ascontiguousarray(
                a.reshape(NCP, 2, P, NJ, TQ).transpose(3, 2, 0, 1, 4)
            )
        in_maps.append(
            {"x8t": pack(x8), "xr8t": pack(xr8), "w8all": w8all, "ebp": ebp}
        )

    nc = _get_nc()
    from concourse import bass2jax

    try:
        results = bass2jax.run_bass_via_pjrt(nc, in_maps, n_cores=NCORES)
    except Exception:
        # transient NRT execution errors occasionally wedge a core; one retry
        import time as _time

        _time.sleep(2.0)
        results = bass2jax.run_bass_via_pjrt(nc, in_maps, n_cores=NCORES)

    out = np.empty((B, T, H), dtype=np.float32)
    for b in range(NCORES):
        outt = results[b]["outt"].astype(np.float32)     # [NJ, H, TQ], x WS
        rsum = results[b]["rsum"].astype(np.float32)     # [NJ, 1, TQ]
        norm = outt / (rsum[:, None, 0, :] * np.float32(WS))
        out[b] = norm.transpose(0, 2, 1).reshape(T, H)
    return out


# revision 38
# speedup vs baseline: 1.0157x; 1.0157x over previous
"""Trainium2 Bass kernel for nn_ExactSpectralHead (sparse resonance attention).

Reference computation (per batch element b):
    q = x @ Wq.T; k = x @ Wk.T; v = x @ Wv.T          # [T, H]
    s = (q @ k.T) * C**-0.5 + resonance_bias          # [T, T]
    s = where(allowed, s, -inf); p = softmax(s, -1)
    out = p @ v                                        # [T, H]

Strategy (8 NeuronCores, data-parallel over batch B=8, one b per core):
  - Host folds bias+mask into EB = exp(bias) * allowed (exact: exp(log1p(r)) = 1+r),
    so p_raw = exp(s_qk * scale) * EB with no -inf handling and exact zeros.
    Scores are bounded (|s|<~5), so no max-subtraction is needed; normalization
    (division by the row sum) is done on the HOST from the unnormalized PV
    output plus a row-sum computed on-device via a ones-matmul.
  - Everything is computed in a transposed layout so that every matmul contracts
    over the partition dim with zero on-device transposes:
      xT [C, T] (host-transposed), QT/KT = W.T^T @ xT -> [H, T],
      ST[tk, tq] = KT.T @ QT, PT = exp(ST*scale) * EBT,
      OT[h, tq] += V[tk,:].T @ PT[tk, tq]   (V in natural [T, H] layout),
      rowsum[tq] = ones.T @ sum_i PT_i, out = (OT / rowsum).T (host).
  - Q/K projections use fp8e4 inputs with DoubleRow matmuls (two 128-deep
    contraction chunks per pass -> 2x PE throughput). The score noise this
    introduces is ~0.3% absolute on s (scores are tiny vs the bias), well
    inside the 2e-2 tolerance. V stays bf16 (fp8 V noise would land ~1:1 on
    the output).
  - bf16 matmul inputs elsewhere (1 col/cycle on the PE), fp32 PSUM accum.
  - Causal block skipping: tiles with tk_chunk > tq_block are never touched.
  - The PE p-state ramps to 2.4GHz only after ~3us of continuous work, so a
    burst of dummy warmup matmuls runs during the initial DMA wait.
"""

import sys

sys.path.insert(0, "/opt/trn_rl_repo")

import numpy as np
import ml_dtypes

import concourse.bass as bass
import concourse.tile as tile
import concourse.mybir as mybir

# ----------------------------------------------------------------------------
# Workaround for walrus codegen "Too many sync wait commands" on the
# TileContext tail Drain: split the global-clock sem waits across multiple SP
# NOP instructions instead of attaching them all to the single Drain.
from concourse.vector_clock import ScopedClock, VectorClock


def _split_drain_and_barrier(self, tick_clock, wait_clock):
    """Cheap kernel tail: per-proc sem waits split across SP NOPs (walrus
    one-wait-per-instruction limit), then a regular-semaphore all-engine
    completion barrier (the stock EVSEM butterfly costs ~1.5-4us per hop),
    then GpSimd clears the tile semaphores. The next NEFF execution cannot
    start until every engine stream (including the clear) retires, so no
    trailing barrier is needed."""
    import concourse.mybir as _mybir

    nc = self.nc
    gc = tick_clock.global_clock
    n = len(gc)
    for p in range(n):
        t = gc[p]
        if t > 0:
            nop = nc.sync.nop(nofuse=True, hint=f"drain_wait_{p}")
            vc = VectorClock([t if i == p else 0 for i in range(n)])
            wait_clock.add_sem_waits(nop.ins, ScopedClock({None: vc}))

    tail_sem = nc.alloc_semaphore("tile_tail_sem")
    n_signals = 0
    for etype, eng in nc.engines.items():
        if etype == _mybir.EngineType.Pool:
            continue
        eng.drain(fusable=False)
        eng.sem_inc(tail_sem, 1)
        n_signals += 1
    nc.gpsimd.wait_ge(tail_sem, n_signals)
    assert self.sems is not None
    popped = nc._tile_sem_poison_stack.pop()
    assert popped is self._sem_poison
    nc.clear_and_free_semaphores(list(self.sems.allocated().values()))
    nc.gpsimd.sem_clear(range(tail_sem.num, tail_sem.num + 1))


tile.TileContext._drain_and_barrier = _split_drain_and_barrier
# ----------------------------------------------------------------------------

def _split_excess_waits(nc, max_waits=1):
    """Walrus codegen in this toolchain supports only one sem-wait per
    instruction; hoist excess waits onto preceding same-engine NOPs."""
    for f in nc.m.functions:
        for bb in f.blocks:
            new = []
            changed = False
            for inst in bb.instructions:
                if isinstance(inst, mybir.InstEventSemaphore):
                    # EventSemaphore ops measure ~3-5us on HW; their barrier
                    # semantics live entirely in sync_info (regular sems), so
                    # NoOps with the same sync_info are equivalent and fast.
                    # Waits and updates go on separate NoOps (wait first) to
                    # satisfy the no_semaphore_value_conflict ISA check.
                    si = inst.sync_info
                    changed = True
                    w = list(si.on_wait) if si else []
                    u = list(si.on_update) if si else []
                    if w:
                        new.append(
                            mybir.InstNoOp(
                                name=f"{inst.name}-wait",
                                engine=inst.engine,
                                bass_nofuse=True,
                                sync_info=mybir.SyncInfo(on_wait=w, on_update=[]),
                            )
                        )
                    new.append(
                        mybir.InstNoOp(
                            name=inst.name,
                            engine=inst.engine,
                            bass_nofuse=True,
                            sync_info=mybir.SyncInfo(on_wait=[], on_update=u),
                        )
                    )
                    continue
                si = inst.sync_info
                waits = list(si.on_wait) if si is not None else []
                if len(waits) > max_waits:
                    changed = True
                    excess, keep = waits[:-max_waits], waits[-max_waits:]
                    for k, w in enumerate(excess):
                        new.append(
                            mybir.InstNoOp(
                                name=f"{inst.name}-w{k}",
                                engine=inst.engine,
                                bass_nofuse=True,
                                sync_info=mybir.SyncInfo(on_wait=[w], on_update=[]),
                            )
                        )
                    inst.sync_info = mybir.SyncInfo(
                        on_wait=keep, on_update=list(si.on_update)
                    )
                new.append(inst)
            if changed:
                bb.instructions = new


B, T, C, H = 8, 2048, 1024, 128
NCORES = 8
QS = 16.0                # host-side Wq/Wk scale (lifts fp8 weights out of subnormals)
WS = 64.0                # host-side Wv scale (ditto; host divides the output by it)
SCALE = float(C) ** -0.5 / (QS * QS)   # exp() scale absorbs the q/k scaling
P = 128
TQ = 512                 # tq block width (matmul moving dim)
NJ = T // TQ             # 4 tq blocks
NC_CHUNK = C // P        # 8 contraction chunks over channels
NCP = NC_CHUNK // 2      # 4 DoubleRow c-chunk pairs
NK = T // P              # 16 tk chunks
BF16 = mybir.dt.bfloat16
FP8 = mybir.dt.float8e4
F32 = mybir.dt.float32
DR = mybir.MatmulPerfMode.DoubleRow

_nc_cache = None


def _build_nc():
    nc = bass.Bass()
    # fp8 DoubleRow pack of xT: x8[j, p, cp, i, q] = xT[(2cp+i)*128+p, j*TQ+q]
    # (partition-major so each DMA moves 4KB contiguous per partition)
    x8t = nc.declare_dram_parameter("x8t", [NJ, P, NCP, 2, TQ], FP8, isOutput=False)
    # fp8 residual pack: xr8 = fp8(xT - fp8(xT)); x8+xr8 represents x to ~0.13%
    xr8t = nc.declare_dram_parameter("xr8t", [NJ, P, NCP, 2, TQ], FP8, isOutput=False)
    # fp8 DoubleRow packs of all weights (one DMA), partition-major:
    # w8all[p, s, cp, i, h] = W_s[h, (2cp+i)*128+p], s = (Wq, Wk, Wv, Wv_resid)
    w8all = nc.declare_dram_parameter("w8all", [P, 4, NCP, 2, H], FP8, isOutput=False)
    # ebT packed partition-major, j-grouped quads (quad (j,q4) at slot off_j+q4):
    # ebp[p, off_j + q4, k, q] = EB.T[128*(4*q4+k)+p, j*TQ+q]; one DMA per j.
    # bf16: a fp8 operand halves the DVE mul rate, so spend the DMA bytes
    ebp = nc.declare_dram_parameter("ebp", [P, 10, 4, TQ], BF16, isOutput=False)
    # unnormalized PV output, bf16: [j, H, TQ] (host divides by rowsum + reassembles)
    outt = nc.declare_dram_parameter("outt", [NJ, H, TQ], BF16, isOutput=True)
    # rowsums, fp32: [j, 1, TQ]
    rsum = nc.declare_dram_parameter("rsum", [NJ, 1, TQ], F32, isOutput=True)

    with tile.TileContext(nc) as tc:
        with (
            tc.tile_pool(name="const", bufs=1) as const,
            tc.tile_pool(name="qkv_psum", bufs=2, space="PSUM") as qkv_psum,
            tc.tile_pool(name="st_psum", bufs=2, space="PSUM") as st_psum_pool,
            tc.tile_pool(name="ot_psum", bufs=1, space="PSUM") as ot_psum_pool,
            tc.tile_pool(name="rs_psum", bufs=1, space="PSUM") as rs_psum_pool,
            tc.tile_pool(name="pt", bufs=NK // 2 + 4) as pt_pool,
            tc.tile_pool(name="eb", bufs=1) as eb_pool,
            tc.tile_pool(name="outs", bufs=4) as out_pool,
        ):
            # ---------- persistent SBUF tensors ----------
            # warmup garbage tile: ramp the PE p-state while DMAs land.
            warm = const.tile([P, P], BF16, tag="warm", name="warm_sb")
            warm_ps = qkv_psum.tile([P, TQ], F32, tag="qkvps", name="qkvps")
            nc.vector.memset(warm[:], 0.0)
            for i in range(56):
                nc.tensor.matmul(
                    warm_ps[:, :P], lhsT=warm[:], rhs=warm[:],
                    start=True, stop=True, skip_group_check=True,
                )

            w8all_sb = const.tile([P, 4, NCP, 2, H], FP8, tag="w8all", name="w8all_sb")
            x8_sb = const.tile([P, NJ, NCP, 2, TQ], FP8, tag="x8", name="x8_sb")
            xr8_sb = const.tile([P, NJ, NCP, 2, TQ], FP8, tag="xr8", name="xr8_sb")
            # per-j eb tiles (all quads of a block in one tile, one DMA)
            ebjs = [
                eb_pool.tile([P, (j + 1) * 4, TQ], BF16, tag=f"eb{j}", name=f"eb{j}")
                for j in range(NJ)
            ]
            EBOFF = [0, 1, 3, 6]

            # ---------- all DMAs issued up-front, in consumption order.   ----------
            # Few, large transfers: the DMA queues have a multi-us fixed cost
            # per transfer. Per-queue lists are in global consumption order;
            # start-critical bulk rides the fast sync/gpsimd queues, the slow
            # scalar queue gets only mid/late items.
            def load_x8(j, eng):
                eng.dma_start(x8_sb[:, j], x8t[j])

            def load_xr8(j, eng):
                eng.dma_start(xr8_sb[:, j], xr8t[j])

            def load_eb(j, eng):
                eng.dma_start(ebjs[j][:], ebp[:, EBOFF[j]:EBOFF[j] + j + 1])

            # need-ordered per queue; eb2 is issued later from the scalar
            # stream (inside the j=0 attention loop) so its bulk does not
            # steal early bandwidth. scalar also carries the outputs.
            # The start-critical path (w8q + x8j0 for the first QT matmuls)
            # is split into small leading transfers.
            # x8j0 split across all three queues: early HBM bandwidth is
            # fair-shared per queue chip-wide, so more queues = faster landing
            nc.scalar.dma_start(w8all_sb[:, 0:1], w8all[:, 0:1])  # w8q, 64KB
            nc.sync.dma_start(x8_sb[:, 0, 0:2], x8t[0, :, 0:2])   # 256KB
            nc.scalar.dma_start(x8_sb[:, 0, 2], x8t[0, :, 2])     # 128KB
            nc.gpsimd.dma_start(x8_sb[:, 0, 3], x8t[0, :, 3])     # 128KB
            nc.gpsimd.dma_start(w8all_sb[:, 1:4], w8all[:, 1:4])  # wk/wv/wvr
            load_xr8(0, nc.gpsimd)                      # 512KB
            load_eb(0, nc.sync)                         # 512KB
            load_x8(1, nc.gpsimd)                       # 512KB
            load_xr8(1, nc.sync)                        # 512KB
            load_eb(1, nc.gpsimd)                       # 1MB
            load_x8(2, nc.sync)                         # 512KB
            load_xr8(2, nc.gpsimd)                      # 512KB
            load_x8(3, nc.gpsimd)                       # 512KB
            load_xr8(3, nc.sync)                        # 512KB
            load_eb(3, nc.gpsimd)                       # 2MB

            QT_sb = const.tile([P, T], BF16, tag="QT", name="QT_sb")
            KT_sb = const.tile([P, T], BF16, tag="KT", name="KT_sb")
            v_sb = const.tile([P, NK, H], BF16, tag="V", name="v_sb")
            ones_sb = const.tile([P, P], BF16, tag="ones", name="ones_sb")
            nc.vector.memset(ones_sb[:], 1.0)
            ones8_sb = const.tile([P, 2, P], FP8, tag="ones8", name="ones8_sb")
            nc.vector.memset(ones8_sb[:], 1.0)

            # ---------- interleaved per tq-block: V_(4j..4j+3), ATT_j; ----------
            # ---------- QT/KT for block j+1 are emitted inside block j's ----------
            # ---------- trailing section to hide the exp/mul chase there ----------
            def emit_qkt(j):
                for s, dst in ((0, QT_sb), (1, KT_sb)):
                    ps = qkv_psum.tile([P, TQ], F32, tag="qkvps", name="qkvps")
                    for cp in range(NCP):
                        nc.tensor.matmul(
                            ps[:],
                            lhsT=w8all_sb[:, s, cp],
                            rhs=x8_sb[:, j, cp],
                            start=(cp == 0),
                            stop=(cp == NCP - 1),
                            perf_mode=DR,
                        )
                    nc.vector.tensor_copy(dst[:, j * TQ:(j + 1) * TQ], ps[:])

            def emit_v(j):
                # V chunks 4j .. 4j+3: compensated fp8 DoubleRow,
                # v = (x8 + xr8) @ w8v + x8 @ w8vr  (~0.15% accurate)
                for mq in range(4):
                    m = 4 * j + mq
                    msl = slice(mq * P, (mq + 1) * P)
                    ps = qkv_psum.tile([P, TQ], F32, tag="qkvps", name="qkvps")
                    first = True
                    for xsb, ws in ((x8_sb, 2), (xr8_sb, 2), (x8_sb, 3)):
                        for cp in range(NCP):
                            nc.tensor.matmul(
                                ps[:, :H],
                                lhsT=xsb[:, j, cp, :, msl],
                                rhs=w8all_sb[:, ws, cp],
                                start=first,
                                stop=(ws == 3 and cp == NCP - 1),
                                perf_mode=DR,
                            )
                            first = False
                    nc.vector.tensor_copy(v_sb[:, m, :], ps[:, :H])

            emit_qkt(0)
            for j in range(NJ):
                emit_v(j)

                # attention for tq block j (causal: tk chunks 0 .. 4j+3).
                # st pairs are emitted ahead; ot/rs matmuls for pair p are
                # emitted after st pair p+2, so the PE never waits on the
                # ACT(exp) -> DVE(mul) chase.
                n_i = 4 * j + 4
                n2 = n_i // 2
                ot = ot_psum_pool.tile([P, TQ], F32, tag="ot", name="ot")
                rs = rs_psum_pool.tile([P, TQ], F32, tag="rs", name="rs")
                pts = []
                pt8s = []

                def emit_ot(p):
                    for k in range(2):
                        i = 2 * p + k
                        o = coff(i)
                        nc.tensor.matmul(
                            ot[:, o:],
                            lhsT=v_sb[:, i, :],
                            rhs=pts[p][:, k, o:],
                            start=(i == 0),
                            stop=(i == n_i - 1),
                            skip_group_check=True,
                        )

                def emit_rs(p):
                    if p >= n2 - 2:
                        # diagonal quad in bf16: sparse queries (prime-ish
                        # positions) have all their softmax mass here, and a
                        # num/den precision mismatch would not cancel for them
                        for k in range(2):
                            i = 2 * p + k
                            o = coff(i)
                            nc.tensor.matmul(
                                rs[:, o:],
                                lhsT=ones_sb[:],
                                rhs=pts[p][:, k, o:],
                                start=(i == 0),
                                stop=(i == n_i - 1),
                                skip_group_check=True,
                            )
                    else:
                        # off-diagonal pairs: fp8 DoubleRow, one pass per pair;
                        # their noise averages out over the wide support
                        nc.tensor.matmul(
                            rs[:],
                            lhsT=ones8_sb[:],
                            rhs=pt8s[p][:],
                            start=(p == 0),
                            stop=False,
                            perf_mode=DR,
                            skip_group_check=True,
                        )

                def emit_otrs(p):
                    emit_ot(p)
                    emit_rs(p)

                def coff(i):
                    # causal offset: tk chunk i only attends tq >= (i-4j)*128
                    return max(0, (i - 4 * j) * P)

                for p in range(n2):
                    st2 = st_psum_pool.tile([P, 2, TQ], F32, tag="st", name="st2")
                    for k in range(2):
                        i = 2 * p + k
                        o = coff(i)
                        nc.tensor.matmul(
                            st2[:, k, o:],
                            lhsT=KT_sb[:, i * P:(i + 1) * P],
                            rhs=QT_sb[:, j * TQ + o:(j + 1) * TQ],
                            start=True,
                            stop=True,
                        )
                    pt = pt_pool.tile([P, 2, TQ], BF16, tag="pt", name="pt")
                    if p == n2 - 1:
                        for k in range(2):
                            nc.scalar.activation(
                                pt[:, k, :], st2[:, k, :],
                                mybir.ActivationFunctionType.Exp, scale=SCALE,
                            )
                            nc.vector.tensor_mul(
                                pt[:, k, :], pt[:, k, :],
                                ebjs[j][:, 2 * p + k, :],
                            )
                    else:
                        nc.scalar.activation(
                            pt[:], st2[:], mybir.ActivationFunctionType.Exp,
                            scale=SCALE,
                        )
                        nc.vector.tensor_mul(
                            pt[:], pt[:], ebjs[j][:, 2 * p:2 * p + 2, :]
                        )
                    pts.append(pt)
                    if p < n2 - 2:
                        pt8 = pt_pool.tile([P, 2, TQ], FP8, tag="pt8", name="pt8")
                        nc.vector.tensor_copy(pt8[:], pt[:])
                        pt8s.append(pt8)
                    else:
                        pt8s.append(None)
                    if j == 0 and p == 1:
                        # artificial WAW dep delays the bulky eb2 transfer so it
                        # cannot steal bandwidth from the start-critical loads
                        nc.vector.tensor_copy(
                            ebjs[2][0:1, 0:1, 0:1], pt[0:1, 0:1, 0:1]
                        )
                        load_eb(2, nc.scalar)           # 1.5MB, deferred issue
                    if p >= 2:
                        emit_otrs(p - 2)
                # next block's projections slot in while the trailing
                # exp/mul chain for the last pairs drains
                if j + 1 < NJ:
                    emit_qkt(j + 1)
                # trailing pairs: rowsum matmuls first so the rs DMA can go
                # out while the PE finishes the ot matmuls
                for p in range(max(0, n2 - 2), n2):
                    emit_rs(p)
                for p in range(max(0, n2 - 2), n2):
                    emit_ot(p)

                # rowsum out (all 128 psum partitions hold the same sums; DMA row 0)
                rsb = out_pool.tile([1, TQ], F32, tag="rsb", name="rsb")
                nc.vector.tensor_copy(rsb[:], rs[0:1, :])
                nc.scalar.dma_start(rsum[j][:], rsb[:])
                # unnormalized OT out, bf16; last block in halves to overlap
                # the copy with the DMA on the tail
                otb = out_pool.tile([P, TQ], BF16, tag="otb", name="otb")
                if j == NJ - 1:
                    HQ = TQ // 2
                    for hh in range(2):
                        sl = slice(hh * HQ, (hh + 1) * HQ)
                        nc.vector.tensor_copy(otb[:, sl], ot[:, sl])
                        nc.scalar.dma_start(outt[j][:, sl], otb[:, sl])
                else:
                    nc.vector.tensor_copy(otb[:], ot[:])
                    nc.scalar.dma_start(outt[j][:], otb[:])

    _split_excess_waits(nc)
    return nc


def _get_nc():
    global _nc_cache
    if _nc_cache is None:
        _nc_cache = _build_nc()
    return _nc_cache


def kernel(x, Wq, Wk, Wv, resonance_bias, allowed):
    x = np.asarray(x, dtype=np.float32)
    Wq = np.asarray(Wq, dtype=np.float32)
    Wk = np.asarray(Wk, dtype=np.float32)
    Wv = np.asarray(Wv, dtype=np.float32)
    resonance_bias = np.asarray(resonance_bias, dtype=np.float32)
    allowed = np.asarray(allowed)

    bf16 = ml_dtypes.bfloat16
    f8 = ml_dtypes.float8_e4m3
    eb = np.exp(resonance_bias) * allowed  # exp(log1p(r))*mask = (1+r)*mask, exact
    ebT = eb.T.astype(bf16)                              # [tk, tq]
    # partition-major, j-grouped quads: ebp[p, EBOFF[j]+q4, k, q]
    #   = EB.T[128*(4*q4+k)+p, j*TQ+q]
    ebq5 = ebT.reshape(NK // 4, 4, P, NJ, TQ)            # [q4, k, p, j, q]
    ebp = np.empty((P, 10, 4, TQ), dtype=bf16)
    EBOFF = [0, 1, 3, 6]
    for j in range(NJ):
        for q4 in range(j + 1):
            ebp[:, EBOFF[j] + q4] = ebq5[q4, :, :, j, :].transpose(1, 0, 2)
    ebp = np.ascontiguousarray(ebp)
    # fp8 DoubleRow weight packs, partition-major, (q, k, v, v_resid) stacked:
    # w8all[p, s, cp, i, h] = W_s[h, (2cp+i)*128+p]. Weights are pre-scaled on
    # the host (QS for q/k, WS for v) so their fp8 encodings avoid the
    # subnormal range; the v residual pack makes v accurate to ~4e-4.
    wv64 = Wv * WS
    wv8 = wv64.astype(f8).astype(np.float32)
    wvr = wv64 - wv8
    w8all = np.ascontiguousarray(
        np.stack(
            [W.T.reshape(NCP, 2, P, H).transpose(2, 0, 1, 3).astype(f8)
             for W in (Wq * QS, Wk * QS, wv8, wvr)],
            axis=1,
        )
    )

    in_maps = []
    for b in range(NCORES):
        xT = x[b].T  # [C, T] fp32
        # x8[j, p, cp, i, q] = xT[(2cp+i)*128+p, j*TQ+q]; xr8 = fp8 residual
        x8 = xT.astype(f8)
        xr8 = (xT - x8.astype(np.float32)).astype(f8)
        def pack(a):
            return np.ascontiguousarray(
                a.reshape(NCP, 2, P, NJ, TQ).transpose(3, 2, 0, 1, 4)
            )
        in_maps.append(
            {"x8t": pack(x8), "xr8t": pack(xr8), "w8all": w8all, "ebp": ebp}
        )

    nc = _get_nc()
    from concourse import bass2jax

    try:
        results = bass2jax.run_bass_via_pjrt(nc, in_maps, n_cores=NCORES)
    except Exception:
        # transient NRT execution errors occasionally wedge a core; one retry
        import time as _time

        _time.sleep(2.0)
        results = bass2jax.run_bass_via_pjrt(nc, in_maps, n_cores=NCORES)

    out = np.empty((B, T, H), dtype=np.float32)
    for b in range(NCORES):
        outt = results[b]["outt"].astype(np.float32)     # [NJ, H, TQ], x WS
        rsum = results[b]["rsum"].astype(np.float32)     # [NJ, 1, TQ]
        norm = outt / (rsum[:, None, 0, :] * np.float32(WS))
        out[b] = norm.transpose(0, 2, 1).reshape(T, H)
    return out
